# revision 1
# baseline (speedup 1.0000x reference)
"""Trainium2 Bass kernel for a TF-style GRU + sigmoid projection.

Reference computation (B=32, T=2048, D=H=OUT=256):
    ru  = sigmoid([x_t, h] @ Wg + bg);  r, u = split(ru)
    c   = tanh([x_t, r*h] @ Wc + bc)
    h'  = u*h + (1-u)*c
    out = sigmoid(H @ Wp + bp)          # H = all h_t

Strategy: data-parallel over batch (8 cores x 4 sequences).  Everything on
chip lives "hidden-major" (transposed): tensors are [hidden(128-part) x
(k-tile, time*batch)] so per-step elementwise/activation ops use all 128
lanes.  The x-dependent halves of the gate/candidate matmuls are precomputed
per 64-step chunk directly into PSUM banks; the sequential loop accumulates
the h-dependent matmuls on top (start=False), so no explicit adds are needed.
Projection runs per chunk, overlapped with the recurrence.
"""

import numpy as np

B, T, D = 32, 2048, 256
H, OUT = 256, 256
NCORES = 8
BLOC = B // NCORES  # 4 sequences per core
CHUNK = 64          # steps per PSUM staging chunk

_cache = {}


def _build(T_, C_):
    import concourse.bacc as bacc
    import concourse.mybir as mybir
    from concourse.tile import TileContext

    f32 = mybir.dt.float32
    bf16 = mybir.dt.bfloat16
    AF = mybir.ActivationFunctionType
    ALU = mybir.AluOpType

    TB = T_ * BLOC
    CB = C_ * BLOC
    nchunks = T_ // C_

    nc = bacc.Bacc("TRN2", target_bir_lowering=False, debug=False)

    xT_d = nc.declare_dram_parameter("xT", [2, 128, TB], bf16, isOutput=False)
    wgx_d = nc.declare_dram_parameter("Wgx", [2, 128, 512], bf16, isOutput=False)
    wgh_d = nc.declare_dram_parameter("Wgh", [2, 128, 512], bf16, isOutput=False)
    wcx_d = nc.declare_dram_parameter("Wcx", [2, 128, 256], bf16, isOutput=False)
    wch_d = nc.declare_dram_parameter("Wch", [2, 128, 256], bf16, isOutput=False)
    wp_d = nc.declare_dram_parameter("Wp", [2, 128, 256], bf16, isOutput=False)
    bg_d = nc.declare_dram_parameter("bg", [1, 512], bf16, isOutput=False)
    bc_d = nc.declare_dram_parameter("bc", [1, 256], bf16, isOutput=False)
    bp_d = nc.declare_dram_parameter("bp", [1, 256], bf16, isOutput=False)
    outT_d = nc.declare_dram_parameter("outT", [2, 128, TB], f32, isOutput=True)

    with TileContext(nc) as tc:
        with (
            tc.tile_pool(name="const", bufs=1) as const,
            tc.tile_pool(name="small", bufs=3) as small,
            tc.tile_pool(name="outp", bufs=3) as outp,
            tc.tile_pool(name="psg", bufs=2, space="PSUM") as psg,
            tc.tile_pool(name="psp", bufs=2, space="PSUM") as psp,
        ):
            xT = const.tile([128, 2, TB], bf16)
            hT = const.tile([128, 2, TB], bf16)
            wgx = const.tile([128, 2, 512], bf16)
            wgh = const.tile([128, 2, 512], bf16)
            wcx = const.tile([128, 2, 256], bf16)
            wch = const.tile([128, 2, 256], bf16)
            wp = const.tile([128, 2, 256], bf16)
            bg = const.tile([1, 512], bf16)
            bc = const.tile([1, 256], bf16)
            bp = const.tile([1, 256], bf16)
            ones = const.tile([1, CB], bf16)
            h0b = const.tile([128, 2, BLOC], bf16)

            for k in range(2):
                nc.sync.dma_start(out=xT[:, k, :], in_=xT_d[k])
                nc.sync.dma_start(out=wgx[:, k, :], in_=wgx_d[k])
                nc.sync.dma_start(out=wgh[:, k, :], in_=wgh_d[k])
                nc.sync.dma_start(out=wcx[:, k, :], in_=wcx_d[k])
                nc.sync.dma_start(out=wch[:, k, :], in_=wch_d[k])
                nc.sync.dma_start(out=wp[:, k, :], in_=wp_d[k])
            nc.sync.dma_start(out=bg[:], in_=bg_d[:])
            nc.sync.dma_start(out=bc[:], in_=bc_d[:])
            nc.sync.dma_start(out=bp[:], in_=bp_d[:])
            nc.vector.memset(ones[:], 1.0)
            nc.vector.memset(h0b[:], 0.0)

            def precompute(c):
                """Stage Gx/Cx (+bias) for chunk c into fresh PSUM tiles.
                Returns the tiles and thunks for the staging matmuls, which
                the step loop spreads across the chunk."""
                cols = slice(c * CB, (c + 1) * CB)
                pr = psg.tile([128, 2, C_, BLOC], f32, tag="pr")
                pu = psg.tile([128, 2, C_, BLOC], f32, tag="pu")
                pc = psg.tile([128, 2, C_, BLOC], f32, tag="pc")
                thunks = []

                # start=True clears the has_written bits of the WHOLE bank, so
                # it must be used exactly once per PSUM tile (first touch).
                def stage(dst, mi, w, k, m, start):
                    def run():
                        return [nc.tensor.matmul(
                            dst[:, mi, :, :],
                            w[:, k, m:m + 128],
                            xT[:, k, cols],
                            start=start,
                            stop=False,
                        )]
                    return run

                def stage_bias(dst, mi, brow, m):
                    def run():
                        return [nc.tensor.matmul(
                            dst[:, mi, :, :],
                            brow[:1, m:m + 128],
                            ones[:1, :],
                            start=False,
                            stop=False,
                        )]
                    return run

                for mi in range(2):
                    for dst, w, brow, moff in (
                        (pr, wgx, bg, 0),
                        (pu, wgx, bg, 256),
                        (pc, wcx, bc, 0),
                    ):
                        m = moff + mi * 128
                        for k in range(2):
                            thunks.append(
                                stage(dst, mi, w, k, m, k == 0 and mi == 0)
                            )
                        thunks.append(stage_bias(dst, mi, brow, m))
                return (pr, pu, pc), thunks

            def gate_mms(dst_r, dst_u, jn, operand, stop):
                """Accumulate Wgh @ operand into step jn's gate PSUM slices."""
                for dst, moff in ((dst_r, 0), (dst_u, 256)):
                    for mi in range(2):
                        for k in range(2):
                            nc.tensor.matmul(
                                dst[:, mi, jn, :],
                                wgh[:, k, moff + mi * 128:moff + (mi + 1) * 128],
                                operand[:, k, :],
                                start=False,
                                stop=(stop and k == 1),
                            )

            def step(pr, pu, pc, j, t, h_prev_b, nxt_dst, prev_insts=None):
                # By this point the gate pre-activations for step j already
                # hold Gx + bg + Wgh@(u*h) + Wgh@((1-u)*c)  (the h-dependent
                # parts were accumulated by the previous step, split by
                # linearity so the u*h half ran off the critical path).
                r_sb = small.tile([128, 2, BLOC], f32, tag="r")
                nc.scalar.activation(r_sb[:], pr[:, :, j, :], AF.Sigmoid)
                rh = small.tile([128, 2, BLOC], bf16, tag="rh")
                nc.vector.tensor_mul(rh[:], r_sb[:], h_prev_b[:])
                for mi in range(2):
                    for k in range(2):
                        mm = nc.tensor.matmul(
                            pc[:, mi, j, :],
                            wch[:, k, mi * 128:(mi + 1) * 128],
                            rh[:, k, :],
                            start=False,
                            stop=(k == 1),
                        )
                        if prev_insts and mi == 0 and k == 0:
                            # pin the previous step's staging/projection
                            # matmuls ahead of this step's tensor-engine work
                            # so the scheduler cannot pile them up at chunk
                            # boundaries on the critical path
                            from concourse.bass import _add_dep_helper
                            for pi in prev_insts:
                                _add_dep_helper(
                                    mm.ins, pi.ins, sync=False,
                                    reason="staging before next step",
                                )
                u_sb = small.tile([128, 2, BLOC], f32, tag="u")
                nc.scalar.activation(u_sb[:], pu[:, :, j, :], AF.Sigmoid)
                uh = small.tile([128, 2, BLOC], bf16, tag="uh")
                nc.vector.tensor_mul(uh[:], u_sb[:], h_prev_b[:])
                v = small.tile([128, 2, BLOC], f32, tag="v")
                nc.vector.tensor_scalar(v[:], u_sb[:], -1.0, 1.0, ALU.mult, ALU.add)
                # next step's gate matmuls, u*h part: off the critical path
                if nxt_dst is not None:
                    gate_mms(nxt_dst[0], nxt_dst[1], nxt_dst[2], uh[:], False)
                c_sb = small.tile([128, 2, BLOC], f32, tag="c")
                nc.scalar.activation(c_sb[:], pc[:, :, j, :], AF.Tanh)
                e = small.tile([128, 2, BLOC], bf16, tag="e")
                nc.vector.tensor_mul(e[:], v[:], c_sb[:])
                # next step's gate matmuls, (1-u)*c part: the only piece of
                # the recurrence left on the critical path
                if nxt_dst is not None:
                    gate_mms(nxt_dst[0], nxt_dst[1], nxt_dst[2], e[:], True)
                # h' = e + u*h for the candidate path and the projection
                # (runs in parallel with the gate matmuls above)
                nc.vector.tensor_add(hT[:, :, 4 * t:4 * t + 4], e[:], uh[:])

            def project_thunks(c):
                cols = slice(c * CB, (c + 1) * CB)
                thunks = []
                for mo in range(2):
                    pp = psp.tile([128, CB], f32, tag="pp")

                    def run(pp=pp, mo=mo):
                        insts = []
                        for k in range(2):
                            insts.append(nc.tensor.matmul(
                                pp[:],
                                wp[:, k, mo * 128:(mo + 1) * 128],
                                hT[:, k, cols],
                                start=(k == 0),
                                stop=False,
                            ))
                        insts.append(nc.tensor.matmul(
                            pp[:], bp[:1, mo * 128:(mo + 1) * 128], ones[:1, :],
                            start=False, stop=True,
                        ))
                        ob = outp.tile([128, CB], f32, tag="ob")
                        nc.scalar.activation(ob[:], pp[:], AF.Sigmoid)
                        nc.sync.dma_start(out=outT_d[mo, :, cols], in_=ob[:])
                        return insts
                    thunks.append(run)
                return thunks

            h_prev_b = h0b[:, :, :]
            prev_insts = None
            cur, boot = precompute(0)
            for th in boot:
                th()
            for c in range(nchunks):
                pending = []
                nxt = None
                if c + 1 < nchunks:
                    nxt, pending = precompute(c + 1)
                if c > 0:
                    pending = pending + project_thunks(c - 1)
                pr, pu, pc = cur
                for j in range(C_):
                    t = c * C_ + j
                    if j + 1 < C_:
                        nxt_dst = (pr, pu, j + 1)
                    elif nxt is not None:
                        nxt_dst = (nxt[0], nxt[1], 0)
                    else:
                        nxt_dst = None
                    step(pr, pu, pc, j, t, h_prev_b, nxt_dst, prev_insts)
                    h_prev_b = hT[:, :, 4 * t:4 * t + 4]
                    # spread staging/projection matmuls across the chunk to
                    # fill tensor-engine slack and avoid boundary bubbles
                    prev_insts = pending[j]() if j < len(pending) else None
                for th in pending[C_:]:
                    th()
                if nxt is not None:
                    cur = nxt
            for th in project_thunks(nchunks - 1):
                th()

    # Re-split matmul waits: Tile leaves [ACT-WAR, DVE-RAW] on each in-loop
    # matmul; bacc's move pass would keep the first (stale ACT WAR) on the MM
    # and hoist the LIVE recurrent-h wait onto the LDWEIGHTS, serializing the
    # weight load behind the recurrence.  Instead, put the stale ACT wait on
    # the LDW (it executes early, so the weight load prefetches during the
    # sigmoid/tanh window) and keep the live DVE wait on the MM.
    for blkx in nc.m.functions[0].blocks:
        prev = None
        for inst in blkx.instructions:
            tn = type(inst).__name__
            if (
                tn == "InstMatmult"
                and prev is not None
                and type(prev).__name__ == "InstLdweights"
                and inst.sync_info is not None
                and len(inst.sync_info.on_wait) == 2
                and (prev.sync_info is None or not prev.sync_info.on_wait)
            ):
                w0, w1 = inst.sync_info.on_wait
                names = {str(w0.ant_name or ""), str(w1.ant_name or "")}
                if any(n.startswith("DVE") for n in names) and any(
                    n.startswith("Activation") for n in names
                ):
                    dve = w0 if str(w0.ant_name or "").startswith("DVE") else w1
                    act = w1 if dve is w0 else w0
                    ups = list(inst.sync_info.on_update)
                    pups = (
                        list(prev.sync_info.on_update) if prev.sync_info else []
                    )
                    prev.sync_info = mybir.SyncInfo(on_wait=[act], on_update=pups)
                    inst.sync_info = mybir.SyncInfo(on_wait=[dve], on_update=ups)
            prev = inst

    nc.finalize()
    return nc


def _get_nc(T_, C_):
    key = (T_, C_)
    if key not in _cache:
        _cache[key] = _build(T_, C_)
    return _cache[key]


def _prep_core_inputs(x_core, Wg, bg, Wc, bc, Wp, bp, T_):
    import ml_dtypes

    bf16 = ml_dtypes.bfloat16

    def cast(a):
        return np.ascontiguousarray(a.astype(bf16))

    # hidden-major x: xT[k, p, t*BLOC + b] = x[b, t, k*128+p]
    xT = np.ascontiguousarray(
        x_core.transpose(2, 1, 0).reshape(2, 128, T_ * BLOC)
    )
    return {
        "xT": cast(xT),
        "Wgx": cast(Wg[:256].reshape(2, 128, 512)),
        "Wgh": cast(Wg[256:].reshape(2, 128, 512)),
        "Wcx": cast(Wc[:256].reshape(2, 128, 256)),
        "Wch": cast(Wc[256:].reshape(2, 128, 256)),
        "Wp": cast(Wp.reshape(2, 128, 256)),
        "bg": cast(bg.reshape(1, 512)),
        "bc": cast(bc.reshape(1, 256)),
        "bp": cast(bp.reshape(1, 256)),
    }


def run_gru(x, Wg, bg, Wc, bc, Wp, bp, T_=None, C_=None, trace=False):
    from concourse.bass_utils import run_bass_kernel_spmd

    T_ = T_ or T
    C_ = C_ or CHUNK
    x = np.asarray(x, dtype=np.float32)
    nc = _get_nc(T_, C_)
    in_maps = []
    for core in range(NCORES):
        x_core = x[core * BLOC:(core + 1) * BLOC]
        in_maps.append(_prep_core_inputs(x_core, Wg, bg, Wc, bc, Wp, bp, T_))
    res = run_bass_kernel_spmd(nc, in_maps, list(range(NCORES)), trace=trace)
    outs = []
    for core in range(NCORES):
        oT = res.results[core]["outT"]  # [2, 128, T*BLOC]
        o = oT.reshape(2, 128, T_, BLOC).transpose(3, 2, 0, 1).reshape(BLOC, T_, OUT)
        outs.append(o)
    full = np.concatenate(outs, axis=0).astype(np.float32)
    return full, res


def kernel(x, Wg, bg, Wc, bc, Wp, bp):
    out, _ = run_gru(
        np.asarray(x), np.asarray(Wg), np.asarray(bg), np.asarray(Wc),
        np.asarray(bc), np.asarray(Wp), np.asarray(bp),
    )
    return out



# revision 2
# speedup vs baseline: 7.0446x; 7.0446x over previous
"""Trainium2 Bass kernel for a TF-style GRU + sigmoid projection.

Reference computation (B=32, T=2048, D=H=OUT=256):
    ru  = sigmoid([x_t, h] @ Wg + bg);  r, u = split(ru)
    c   = tanh([x_t, r*h] @ Wc + bc)
    h'  = u*h + (1-u)*c
    out = sigmoid(H @ Wp + bp)          # H = all h_t

Strategy: data-parallel over batch (8 cores x 4 sequences), and
parallel-in-time inside each core via fixed-point (quasi-DEER) sweeps:

    sweep k:  for ALL t in parallel (big matmuls, full engine occupancy):
                  pr,pu = Gx_t + Wgh @ h^{k-1}_{t-1};  r,u = sigmoid
                  c     = tanh(Cx_t + Wch @ (r * h^{k-1}_{t-1}))
                  z     = (u-1)*c            # -(1-u)*c
              then one hardware prefix scan per (k-tile, seq):
                  h^k_t = u_t * h^k_{t-1} - z_t     (DVE tensor_tensor_scan)

The scan makes the u-memory chain exact every sweep; only the gate/candidate
coupling iterates, contracting ~0.37x per sweep.  K=4 sweeps reach ~2e-3
rel L2 (gate is 2e-2).  Sweep 1 (h=0) doubles as the Gx/Cx staging pass.

Everything on chip is hidden-major: [128 partitions = half the hidden dim,
2 k-tiles, cols] with col = seq*2048 + t (t fastest, so the scan can run
along the free dimension per sequence).
"""

import numpy as np

B, T, D = 32, 2048, 256
H, OUT = 256, 256
NCORES = 8
BLOC = B // NCORES      # 4 sequences per core
N = T * BLOC            # 8192 cols, col = b*T + t
CH = 256                # cols per psum chunk
CPB = T // CH           # chunks per sequence
XBLK = 1024             # x-stream / output DMA block
K = 4                   # fixed-point sweeps

_cache = {}


def _build(K_, CH_):
    import concourse.bacc as bacc
    import concourse.mybir as mybir
    from concourse.tile import TileContext

    f32 = mybir.dt.float32
    bf16 = mybir.dt.bfloat16
    AF = mybir.ActivationFunctionType
    ALU = mybir.AluOpType

    CPB_ = T // CH_
    PBLK = XBLK // CH_  # chunks per x/out DMA block

    nc = bacc.Bacc("TRN2", target_bir_lowering=False, debug=False)

    xT_d = nc.declare_dram_parameter("xT", [2, 128, N], bf16, isOutput=False)
    wgx_d = nc.declare_dram_parameter("Wgx", [2, 128, 512], bf16, isOutput=False)
    wgh_d = nc.declare_dram_parameter("Wgh", [2, 128, 512], bf16, isOutput=False)
    wcx_d = nc.declare_dram_parameter("Wcx", [2, 128, 256], bf16, isOutput=False)
    wch_d = nc.declare_dram_parameter("Wch", [2, 128, 256], bf16, isOutput=False)
    wp_d = nc.declare_dram_parameter("Wp", [2, 128, 256], bf16, isOutput=False)
    bg_d = nc.declare_dram_parameter("bg", [1, 512], bf16, isOutput=False)
    bc_d = nc.declare_dram_parameter("bc", [1, 256], bf16, isOutput=False)
    bp_d = nc.declare_dram_parameter("bp", [1, 256], bf16, isOutput=False)
    eye_d = nc.declare_dram_parameter("eye", [128, 128], bf16, isOutput=False)
    outT_d = nc.declare_dram_parameter("outT", [128, 2, N], f32, isOutput=True)

    with TileContext(nc) as tc:
        with (
            tc.tile_pool(name="const", bufs=1) as const,
            tc.tile_pool(name="xc", bufs=2) as xcp,
            tc.tile_pool(name="rsc", bufs=3) as rsc,
            tc.tile_pool(name="csc", bufs=3) as csc,
            tc.tile_pool(name="rhsc", bufs=3) as rhsc,
            tc.tile_pool(name="ub", bufs=2) as ubp,
            tc.tile_pool(name="zb", bufs=2) as zbp,
            tc.tile_pool(name="ob", bufs=2) as obp,
            tc.tile_pool(name="psr", bufs=2, space="PSUM") as psr,
            tc.tile_pool(name="psu", bufs=2, space="PSUM") as psu,
            tc.tile_pool(name="psc", bufs=2, space="PSUM") as psc,
            tc.tile_pool(name="psp", bufs=2, space="PSUM") as psp,
        ):
            gx = const.tile([128, 4, N], bf16)   # Gx+bg, m = [r0,r1,u0,u1]
            cx = const.tile([128, 2, N], bf16)   # Cx+bc
            h = const.tile([128, 2, N], bf16)
            wgx = const.tile([128, 2, 512], bf16)
            wgh = const.tile([128, 2, 512], bf16)
            wcx = const.tile([128, 2, 256], bf16)
            wch = const.tile([128, 2, 256], bf16)
            wp = const.tile([128, 2, 256], bf16)
            eye = const.tile([128, 128], bf16)
            bg = const.tile([1, 512], bf16)
            bc = const.tile([1, 256], bf16)
            bp = const.tile([1, 256], bf16)
            ones = const.tile([1, CH_], bf16)

            for k in range(2):
                nc.sync.dma_start(out=wgx[:, k, :], in_=wgx_d[k])
                nc.sync.dma_start(out=wgh[:, k, :], in_=wgh_d[k])
                nc.sync.dma_start(out=wcx[:, k, :], in_=wcx_d[k])
                nc.sync.dma_start(out=wch[:, k, :], in_=wch_d[k])
                nc.sync.dma_start(out=wp[:, k, :], in_=wp_d[k])
            nc.sync.dma_start(out=eye[:], in_=eye_d[:])
            nc.sync.dma_start(out=bg[:], in_=bg_d[:])
            nc.sync.dma_start(out=bc[:], in_=bc_d[:])
            nc.sync.dma_start(out=bp[:], in_=bp_d[:])
            nc.vector.memset(ones[:], 1.0)

            def sweep1_chunk(b, j, xc, ub_t, zb_t):
                """pr/pu/pc = x-part + bias; store Gx/Cx; u, z for the scan."""
                s = b * T + j * CH_
                off = (j % PBLK) * CH_
                pr = psr.tile([128, 2, CH_], f32, tag="pr")
                pu = psu.tile([128, 2, CH_], f32, tag="pu")
                pc = psc.tile([128, 2, CH_], f32, tag="pc")
                for m in range(4):
                    dst, mi = (pr, m) if m < 2 else (pu, m - 2)
                    for k in range(2):
                        nc.tensor.matmul(
                            dst[:, mi, :],
                            wgx[:, k, m * 128:(m + 1) * 128],
                            xc[:, k, off:off + CH_],
                            start=(k == 0 and mi == 0),
                            stop=False,
                        )
                    nc.tensor.matmul(
                        dst[:, mi, :], bg[:1, m * 128:(m + 1) * 128],
                        ones[:1, :], start=False, stop=(mi == 1),
                    )
                for m in range(2):
                    for k in range(2):
                        nc.tensor.matmul(
                            pc[:, m, :],
                            wcx[:, k, m * 128:(m + 1) * 128],
                            xc[:, k, off:off + CH_],
                            start=(k == 0 and m == 0),
                            stop=False,
                        )
                    nc.tensor.matmul(
                        pc[:, m, :], bc[:1, m * 128:(m + 1) * 128],
                        ones[:1, :], start=False, stop=(m == 1),
                    )
                # stash preactivations for sweeps 2..K
                nc.vector.tensor_scalar(
                    gx[:, 0:2, s:s + CH_], pr[:], 0.0, None, ALU.add)
                nc.vector.tensor_scalar(
                    gx[:, 2:4, s:s + CH_], pu[:], 0.0, None, ALU.add)
                nc.scalar.activation(cx[:, :, s:s + CH_], pc[:], AF.Copy)
                # u, c, z  (r unused: h_prev = 0 this sweep)
                nc.scalar.activation(
                    ub_t[:, :, j * CH_:(j + 1) * CH_], pu[:], AF.Sigmoid)
                c_t = csc.tile([128, 2, CH_], bf16, tag="c")
                nc.scalar.activation(c_t[:], pc[:], AF.Tanh)
                nc.vector.scalar_tensor_tensor(
                    zb_t[:, :, j * CH_:(j + 1) * CH_],
                    ub_t[:, :, j * CH_:(j + 1) * CH_],
                    1.0, c_t[:], ALU.subtract, ALU.mult)

            def sweep_chunk(b, j, ub_t, zb_t):
                """Full chunk: gates/candidate from h^{k-1}, u and z out."""
                s = b * T + j * CH_
                first = (j == 0)
                hs = s if first else s - 1
                ncols = CH_ - 1 if first else CH_
                o0 = 1 if first else 0
                pr = psr.tile([128, 2, CH_], f32, tag="pr")
                pu = psu.tile([128, 2, CH_], f32, tag="pu")
                pc = psc.tile([128, 2, CH_], f32, tag="pc")
                for m in range(4):
                    dst, mi = (pr, m) if m < 2 else (pu, m - 2)
                    nc.tensor.matmul(
                        dst[:, mi, :], eye[:, :], gx[:, m, s:s + CH_],
                        start=(mi == 0), stop=False,
                    )
                for m in range(4):
                    dst, mi = (pr, m) if m < 2 else (pu, m - 2)
                    for k in range(2):
                        nc.tensor.matmul(
                            dst[:, mi, o0:CH_],
                            wgh[:, k, m * 128:(m + 1) * 128],
                            h[:, k, hs:hs + ncols],
                            start=False, stop=(k == 1),
                        )
                r_t = rsc.tile([128, 2, CH_], bf16, tag="r")
                nc.scalar.activation(r_t[:], pr[:], AF.Sigmoid)
                nc.scalar.activation(
                    ub_t[:, :, j * CH_:(j + 1) * CH_], pu[:], AF.Sigmoid)
                rh_t = rhsc.tile([128, 2, CH_], bf16, tag="rh")
                nc.vector.tensor_mul(
                    rh_t[:, :, o0:CH_], r_t[:, :, o0:CH_],
                    h[:, :, hs:hs + ncols])
                for m in range(2):
                    nc.tensor.matmul(
                        pc[:, m, :], eye[:, :], cx[:, m, s:s + CH_],
                        start=(m == 0), stop=False,
                    )
                for m in range(2):
                    for k in range(2):
                        nc.tensor.matmul(
                            pc[:, m, o0:CH_],
                            wch[:, k, m * 128:(m + 1) * 128],
                            rh_t[:, k, o0:CH_],
                            start=False, stop=(k == 1),
                        )
                c_t = csc.tile([128, 2, CH_], bf16, tag="c")
                nc.scalar.activation(c_t[:], pc[:], AF.Tanh)
                nc.vector.scalar_tensor_tensor(
                    zb_t[:, :, j * CH_:(j + 1) * CH_],
                    ub_t[:, :, j * CH_:(j + 1) * CH_],
                    1.0, c_t[:], ALU.subtract, ALU.mult)

            def scans(b, ub_t, zb_t):
                for kk in range(2):
                    nc.vector.tensor_tensor_scan(
                        h[:, kk, b * T:(b + 1) * T],
                        ub_t[:, kk, :], zb_t[:, kk, :],
                        0.0, ALU.mult, ALU.subtract)

            def project(b):
                for jj in range(CPB_):
                    s = b * T + jj * CH_
                    if jj % PBLK == 0:
                        ob = obp.tile([128, 2, XBLK], f32, tag="ob")
                    pp = psp.tile([128, 2, CH_], f32, tag="pp")
                    for mo in range(2):
                        for k in range(2):
                            nc.tensor.matmul(
                                pp[:, mo, :],
                                wp[:, k, mo * 128:(mo + 1) * 128],
                                h[:, k, s:s + CH_],
                                start=(mo == 0 and k == 0), stop=False,
                            )
                        nc.tensor.matmul(
                            pp[:, mo, :], bp[:1, mo * 128:(mo + 1) * 128],
                            ones[:1, :], start=False, stop=(mo == 1),
                        )
                    oo = (jj % PBLK) * CH_
                    nc.scalar.activation(
                        ob[:, :, oo:oo + CH_], pp[:], AF.Sigmoid)
                    if jj % PBLK == PBLK - 1:
                        s0 = b * T + (jj - (PBLK - 1)) * CH_
                        nc.sync.dma_start(
                            out=outT_d[:, :, s0:s0 + XBLK], in_=ob[:])

            # ---- sweep 1 (h=0): staging + first iterate ----
            for b in range(BLOC):
                ub_t = ubp.tile([128, 2, T], bf16, tag="u")
                zb_t = zbp.tile([128, 2, T], bf16, tag="z")
                for j in range(CPB_):
                    if j % PBLK == 0:
                        xc = xcp.tile([128, 2, XBLK], bf16, tag="xc")
                        s0 = b * T + j * CH_
                        for k in range(2):
                            nc.sync.dma_start(
                                out=xc[:, k, :], in_=xT_d[k, :, s0:s0 + XBLK])
                    sweep1_chunk(b, j, xc, ub_t, zb_t)
                scans(b, ub_t, zb_t)

            # ---- sweeps 2..K ----
            for kiter in range(1, K_):
                last = (kiter == K_ - 1)
                for b in range(BLOC):
                    ub_t = ubp.tile([128, 2, T], bf16, tag="u")
                    zb_t = zbp.tile([128, 2, T], bf16, tag="z")
                    for j in range(CPB_):
                        sweep_chunk(b, j, ub_t, zb_t)
                    scans(b, ub_t, zb_t)
                    if last:
                        project(b)

    nc.finalize()
    return nc


def _get_nc(K_, CH_):
    key = (K_, CH_)
    if key not in _cache:
        _cache[key] = _build(K_, CH_)
    return _cache[key]


def _prep_core_inputs(x_core, Wg, bg, Wc, bc, Wp, bp):
    import ml_dtypes

    bf16 = ml_dtypes.bfloat16

    def cast(a):
        return np.ascontiguousarray(a.astype(bf16))

    # hidden-major x: xT[k, p, b*T + t] = x[b, t, k*128+p]
    xT = np.ascontiguousarray(
        x_core.transpose(2, 0, 1).reshape(2, 128, N)
    )
    return {
        "xT": cast(xT),
        "Wgx": cast(Wg[:256].reshape(2, 128, 512)),
        "Wgh": cast(Wg[256:].reshape(2, 128, 512)),
        "Wcx": cast(Wc[:256].reshape(2, 128, 256)),
        "Wch": cast(Wc[256:].reshape(2, 128, 256)),
        "Wp": cast(Wp.reshape(2, 128, 256)),
        "bg": cast(bg.reshape(1, 512)),
        "bc": cast(bc.reshape(1, 256)),
        "bp": cast(bp.reshape(1, 256)),
        "eye": cast(np.eye(128, dtype=np.float32)),
    }


def run_gru(x, Wg, bg, Wc, bc, Wp, bp, K_=None, CH_=None, trace=False):
    from concourse.bass_utils import run_bass_kernel_spmd

    K_ = K_ or K
    CH_ = CH_ or CH
    x = np.asarray(x, dtype=np.float32)
    nc = _get_nc(K_, CH_)
    in_maps = []
    for core in range(NCORES):
        x_core = x[core * BLOC:(core + 1) * BLOC]
        in_maps.append(_prep_core_inputs(x_core, Wg, bg, Wc, bc, Wp, bp))
    res = run_bass_kernel_spmd(nc, in_maps, list(range(NCORES)), trace=trace)
    outs = []
    for core in range(NCORES):
        oT = res.results[core]["outT"]  # [128, 2, N]
        o = (oT.reshape(128, 2, BLOC, T)
             .transpose(2, 3, 1, 0).reshape(BLOC, T, OUT))
        outs.append(o)
    full = np.concatenate(outs, axis=0).astype(np.float32)
    return full, res


def kernel(x, Wg, bg, Wc, bc, Wp, bp):
    out, _ = run_gru(
        np.asarray(x), np.asarray(Wg), np.asarray(bg), np.asarray(Wc),
        np.asarray(bc), np.asarray(Wp), np.asarray(bp),
    )
    return out
